# revision 11
# baseline (speedup 1.0000x reference)
"""Block-causal transformer (B=4, S=1024, D=1024, H=16, DFF=4096, L=4, BLK=64)
on 8 Trainium2 NeuronCores.

Sharding: 8 shards = (4 batch) x (2-way interleaved 64-token blocks).
Core pair (2b, 2b+1) owns batch b; parity p core owns global blocks {2m+p}
(512 tokens). Interleaving balances the block-causal attention load between
the two cores of a pair.

Per core (single SPMD program, per-core differences are input *data* only):
  - activations kept in SBUF in transposed layout (features on partitions)
  - full weights streamed from HBM as bf16 (host pre-transposed/pre-tiled)
  - scores computed transposed S^T[kv, q]; softmax sum comes free from a
    ones-row appended to V; exp on ScalarE with per-partition mask bias
  - LayerNorm stats via ones-vector matmul partition reductions (fp32r)
  - x is exchanged between pair cores with a 2-rank AllGather per layer
    boundary; K/V projections are computed for the full 1024 tokens locally.
"""

import numpy as np
import ml_dtypes

import concourse.bass as bass
import concourse.mybir as mybir
import concourse.tile as tile
from concourse import bacc

P = 128
B, S, D, H, DFF, L, BLK = 4, 1024, 1024, 16, 4096, 4, 64
DH = D // H            # 64
KS = D // P            # 8
FKS = DFF // P         # 32
TOK = 512              # own tokens per core
TT = TOK // P          # 4 q tiles
LN_EPS = 1e-5
MASK_NEG = -60.0
NCORES = 8

F32 = mybir.dt.float32
F32R = mybir.dt.float32r
BF16 = mybir.dt.bfloat16
NPBF = ml_dtypes.bfloat16

RG = [[0, 1], [2, 3], [4, 5], [6, 7]]


def build_program(n_layers=L, n_devices=NCORES):
    nc = bacc.Bacc("TRN2", target_bir_lowering=False, debug=False,
                   num_devices=n_devices)

    # ---- DRAM I/O ----
    xs0_d = nc.dram_tensor("xs0", [P, KS, TOK], F32, kind="ExternalInput")
    xbf0_d = nc.dram_tensor("xbf0", [P, KS, TOK], BF16, kind="ExternalInput")
    xfull0_d = nc.dram_tensor("xfull0", [P, KS, 2 * TOK], BF16, kind="ExternalInput")
    mask_d = nc.dram_tensor("maskc", [P, 4], F32, kind="ExternalInput")
    wqkv_d = nc.dram_tensor("wqkv", [n_layers, 6, P, KS, 512], BF16, kind="ExternalInput")
    wout_d = nc.dram_tensor("wout", [n_layers, KS, P, KS, P], BF16, kind="ExternalInput")
    w1_d = nc.dram_tensor("w1", [n_layers, 8, P, KS, 512], BF16, kind="ExternalInput")
    w2_d = nc.dram_tensor("w2", [n_layers, KS, P, FKS, P], BF16, kind="ExternalInput")
    y_d = nc.dram_tensor("yT", [P, KS, TOK], F32, kind="ExternalOutput")

    with tile.TileContext(nc) as tc:
        _emit(tc, n_layers, xs0_d, xbf0_d, xfull0_d, mask_d,
              wqkv_d, wout_d, w1_d, w2_d, y_d)
    nc.compile()
    return nc


def _emit(tc, n_layers, xs0_d, xbf0_d, xfull0_d, mask_d,
          wqkv_d, wout_d, w1_d, w2_d, y_d):
    nc = tc.nc
    EXP = mybir.ActivationFunctionType.Exp
    RELU = mybir.ActivationFunctionType.Relu
    SQRT = mybir.ActivationFunctionType.Sqrt
    MUL = mybir.AluOpType.mult

    import contextlib
    ctx = contextlib.ExitStack()
    with ctx:
        persist = ctx.enter_context(tc.tile_pool(name="persist", bufs=1))
        state = ctx.enter_context(tc.tile_pool(name="state", bufs=1))
        wpool = ctx.enter_context(tc.tile_pool(name="wpool", bufs=2))
        apool = ctx.enter_context(tc.tile_pool(name="apool", bufs=3))
        small = ctx.enter_context(tc.tile_pool(name="small", bufs=2))
        psum = ctx.enter_context(tc.tile_pool(name="psum", bufs=4, space="PSUM"))
        dram = ctx.enter_context(tc.tile_pool(name="dram", bufs=2, space="DRAM"))

        # ---- persistent tiles ----
        xs = persist.tile([P, KS, TOK], F32, name="xs")          # fp32 state x^T
        ones_col = persist.tile([P, 1], BF16, name="ones_col")
        ones_row_bf = persist.tile([1, P], BF16, name="ones_row_bf")
        mask_sb = persist.tile([P, 4], F32, name="mask_sb")
        eps_sb = persist.tile([1, 1], F32, name="eps_sb")
        nc.vector.memset(ones_col, 1.0)
        nc.vector.memset(ones_row_bf, 1.0)
        nc.vector.memset(eps_sb, LN_EPS)
        nc.sync.dma_start(out=mask_sb, in_=mask_d[:])
        nc.sync.dma_start(out=xs, in_=xs0_d[:])

        agout_prev = None

        for l in range(n_layers):
            # ---------- x buffers for this layer ----------
            if l == 0:
                xbf = state.tile([P, KS, TOK], BF16, tag="xbf", name=f"xbf{l}")
                nc.sync.dma_start(out=xbf, in_=xbf0_d[:])
                xfull = state.tile([P, KS, 2 * TOK], BF16, tag="big", name=f"xfull{l}")
                nc.sync.dma_start(out=xfull, in_=xfull0_d[:])
            else:
                xbf = xbf_next
                xfull = state.tile([P, KS, 2 * TOK], BF16, tag="big", name=f"xfull{l}")
                for r in range(2):
                    nc.sync.dma_start(out=xfull[:, :, r * TOK:(r + 1) * TOK],
                                      in_=agout_prev[r])

            QT = state.tile([P, KS, TOK], BF16, tag="qt", name=f"qt{l}")
            KT = state.tile([P, KS, 2 * TOK], BF16, tag="kt", name=f"kt{l}")
            Vn = state.tile([P, KS, H, DH + 1], BF16, tag="vn", name=f"vn{l}")
            AOT = state.tile([P, KS, TOK], BF16, tag="aot", name=f"aot{l}")
            nc.vector.memset(Vn[:, :, :, DH:DH + 1], 1.0)

            # ---------- QKV projections ----------
            # Q^T (own tokens):  out[qfeat 128, tok 512]
            for c in range(2):
                wt = wpool.tile([P, KS, 512], BF16, tag="wa", name=f"wq{l}_{c}")
                nc.gpsimd.dma_start(out=wt, in_=wqkv_d[l, c])
                for fo in range(4):
                    ps = psum.tile([P, 512], F32, tag="big", name=f"psq{l}_{c}_{fo}")
                    for ks in range(KS):
                        nc.tensor.matmul(ps, lhsT=wt[:, ks, fo * P:(fo + 1) * P],
                                         rhs=xbf[:, ks, :],
                                         start=(ks == 0), stop=(ks == KS - 1))
                    nc.vector.tensor_copy(QT[:, c * 4 + fo, :], ps)
            # K^T (full sequence, slot order)
            for c in range(2, 4):
                wt = wpool.tile([P, KS, 512], BF16, tag="wa", name=f"wk{l}_{c}")
                nc.gpsimd.dma_start(out=wt, in_=wqkv_d[l, c])
                for fo in range(4):
                    for half in range(2):
                        ps = psum.tile([P, 512], F32, tag="big",
                                       name=f"psk{l}_{c}_{fo}_{half}")
                        for ks in range(KS):
                            nc.tensor.matmul(
                                ps, lhsT=wt[:, ks, fo * P:(fo + 1) * P],
                                rhs=xfull[:, ks, half * 512:(half + 1) * 512],
                                start=(ks == 0), stop=(ks == KS - 1))
                        nc.vector.tensor_copy(
                            KT[:, (c - 2) * 4 + fo, half * 512:(half + 1) * 512], ps)
            # V natural [tok 128, feat] (full sequence)
            for c in range(4, 6):
                wt = wpool.tile([P, KS, 512], BF16, tag="wa", name=f"wv{l}_{c}")
                nc.gpsimd.dma_start(out=wt, in_=wqkv_d[l, c])
                for ts in range(8):
                    ps = psum.tile([P, 512], F32, tag="big", name=f"psv{l}_{c}_{ts}")
                    for ks in range(KS):
                        nc.tensor.matmul(ps, lhsT=xfull[:, ks, ts * P:(ts + 1) * P],
                                         rhs=wt[:, ks, :],
                                         start=(ks == 0), stop=(ks == KS - 1))
                    nc.vector.tensor_copy(
                        Vn[:, ts, (c - 4) * 8:(c - 4) * 8 + 8, 0:DH],
                        ps.rearrange("p (h d) -> p h d", d=DH))

            # ---------- attention ----------
            for t in range(TT):
                for h in range(H):
                    po = (h % 2) * DH
                    fh = h // 2
                    n = t + 1
                    ats = []
                    for r in range(2):
                        sc = psum.tile([P, n * P], F32, tag="big",
                                       name=f"sc{l}_{t}_{h}_{r}")
                        for m in range(n):
                            nc.tensor.matmul(
                                sc[:, m * P:(m + 1) * P],
                                lhsT=KT[po:po + DH, fh,
                                        r * 512 + m * P:r * 512 + (m + 1) * P],
                                rhs=QT[po:po + DH, fh, t * P:(t + 1) * P],
                                start=True, stop=True)
                        at = apool.tile([P, n * P], BF16, tag="at",
                                        name=f"at{l}_{t}_{h}_{r}")
                        if t > 0:
                            nc.scalar.activation(at[:, :t * P], sc[:, :t * P], EXP)
                        for qh in range(2):
                            c0 = t * P + qh * 64
                            nc.scalar.activation(
                                at[:, c0:c0 + 64], sc[:, c0:c0 + 64], EXP,
                                bias=mask_sb[:, r * 2 + qh:r * 2 + qh + 1])
                        ats.append(at)
                    ov = psum.tile([DH + 1, P], F32, tag="av", bufs=2, name=f"ov{l}_{t}_{h}")
                    idx = 0
                    for r in range(2):
                        for m in range(n):
                            nc.tensor.matmul(
                                ov, lhsT=Vn[:, r * 4 + m, h, 0:DH + 1],
                                rhs=ats[r][:, m * P:(m + 1) * P],
                                start=(idx == 0), stop=(idx == 2 * n - 1))
                            idx += 1
                    recf = small.tile([1, P], F32, tag="recf", name=f"recf{l}_{t}_{h}")
                    nc.vector.reciprocal(recf, ov[DH:DH + 1, :])
                    rec = small.tile([1, P], BF16, tag="rec", name=f"rec{l}_{t}_{h}")
                    nc.vector.tensor_copy(rec, recf)
                    rb = psum.tile([DH, P], F32, tag="rb", bufs=2, name=f"rb{l}_{t}_{h}")
                    nc.tensor.matmul(rb, lhsT=ones_row_bf[0:1, 0:DH], rhs=rec,
                                     start=True, stop=True)
                    rbs = small.tile([DH, P], F32, tag="rbs", name=f"rbs{l}_{t}_{h}")
                    nc.vector.tensor_copy(rbs, rb)
                    nc.vector.tensor_tensor(
                        AOT[po:po + DH, fh, t * P:(t + 1) * P],
                        ov[0:DH, :], rbs, MUL)

            # ---------- out proj + residual ----------
            xres = state.tile([P, KS, TOK], F32, tag="xres", name=f"xres{l}")
            for ft in range(KS):
                wo = wpool.tile([P, KS, P], BF16, tag="wo", name=f"wo{l}_{ft}")
                nc.gpsimd.dma_start(out=wo, in_=wout_d[l, ft])
                ps = psum.tile([P, 512], F32, tag="big", name=f"pso{l}_{ft}")
                for ks in range(KS):
                    nc.tensor.matmul(ps, lhsT=wo[:, ks, :], rhs=AOT[:, ks, :],
                                     start=(ks == 0), stop=(ks == KS - 1))
                nc.vector.tensor_add(xres[:, ft, :], xs[:, ft, :], ps)

            # ---------- LN1 ----------
            x1bf = state.tile([P, KS, TOK], BF16, tag="x1bf", name=f"x1bf{l}")
            _layernorm(tc, psum, small, state, xres, xs, x1bf,
                       ones_col, ones_row_bf, eps_sb, f"ln1_{l}")

            # ---------- FFN ----------
            HT = state.tile([P, FKS, TOK], BF16, tag="big", name=f"ht{l}")
            for c in range(8):
                wt = wpool.tile([P, KS, 512], BF16, tag="wa", name=f"w1_{l}_{c}")
                nc.gpsimd.dma_start(out=wt, in_=w1_d[l, c])
                for fo in range(4):
                    ps = psum.tile([P, 512], F32, tag="big", name=f"psh{l}_{c}_{fo}")
                    for ks in range(KS):
                        nc.tensor.matmul(ps, lhsT=wt[:, ks, fo * P:(fo + 1) * P],
                                         rhs=x1bf[:, ks, :],
                                         start=(ks == 0), stop=(ks == KS - 1))
                    nc.scalar.activation(HT[:, c * 4 + fo, :], ps, RELU)
            for ft in range(KS):
                w2t = wpool.tile([P, FKS, P], BF16, tag="w2", bufs=2,
                                 name=f"w2_{l}_{ft}")
                nc.gpsimd.dma_start(out=w2t, in_=w2_d[l, ft])
                ps = psum.tile([P, 512], F32, tag="big", name=f"ps2{l}_{ft}")
                for ks in range(FKS):
                    nc.tensor.matmul(ps, lhsT=w2t[:, ks, :], rhs=HT[:, ks, :],
                                     start=(ks == 0), stop=(ks == FKS - 1))
                nc.vector.tensor_add(xres[:, ft, :], xs[:, ft, :], ps)

            # ---------- LN2 ----------
            xbf_next = state.tile([P, KS, TOK], BF16, tag="xbf", name=f"xbf{l + 1}")
            _layernorm(tc, psum, small, state, xres, xs, xbf_next,
                       ones_col, ones_row_bf, eps_sb, f"ln2_{l}")

            # ---------- pair AllGather of x ----------
            if l < n_layers - 1:
                agin = dram.tile([P, KS, TOK], BF16, name=f"agin{l}")
                agout = dram.tile([2, P, KS, TOK], BF16, name=f"agout{l}")
                nc.sync.dma_start(out=agin[:], in_=xbf_next)
                nc.gpsimd.collective_compute(
                    "AllGather", mybir.AluOpType.bypass,
                    replica_groups=RG, ins=[agin.opt()], outs=[agout.opt()])
                agout_prev = agout

        nc.sync.dma_start(out=y_d[:], in_=xs)


def _layernorm(tc, psum, small, state, xres, xs, out_bf,
               ones_col, ones_row_bf, eps_sb, name):
    """xs <- LN(xres) (gamma=1, beta=0), out_bf <- bf16(xs)."""
    nc = tc.nc
    MUL = mybir.AluOpType.mult
    SQRT = mybir.ActivationFunctionType.Sqrt

    xrb = state.tile([P, KS, TOK], BF16, tag="qt", name=f"xrb_{name}")
    nc.vector.tensor_copy(xrb, xres)
    sq = state.tile([P, KS, TOK], BF16, tag="aot", name=f"sq_{name}")
    nc.scalar.activation(sq, xres, mybir.ActivationFunctionType.Square)
    sums = psum.tile([1, TOK], F32, tag="av", bufs=2, name=f"s1_{name}")
    sums2 = psum.tile([1, TOK], F32, tag="rb", bufs=2, name=f"s2_{name}")
    for ks in range(KS):
        nc.tensor.matmul(sums, lhsT=ones_col,
                         rhs=xrb[:, ks, :],
                         start=(ks == 0), stop=(ks == KS - 1))
    for ks in range(KS):
        nc.tensor.matmul(sums2, lhsT=ones_col,
                         rhs=sq[:, ks, :],
                         start=(ks == 0), stop=(ks == KS - 1))
    mus = small.tile([1, TOK], F32, tag="lns", bufs=6, name=f"mu_{name}")
    ex2 = small.tile([1, TOK], F32, tag="lns", bufs=6, name=f"ex2_{name}")
    var = small.tile([1, TOK], F32, tag="lns", bufs=6, name=f"var_{name}")
    a_sb = small.tile([1, TOK], F32, tag="lns", bufs=6, name=f"a_{name}")
    c_sb = small.tile([1, TOK], F32, tag="lns", bufs=6, name=f"c_{name}")
    nc.vector.tensor_scalar_mul(mus, sums, 1.0 / D)
    nc.vector.tensor_scalar_mul(ex2, sums2, 1.0 / D)
    nc.vector.tensor_tensor(var, mus, mus, MUL)
    nc.vector.tensor_sub(var, ex2, var)
    nc.scalar.activation(var, var, SQRT, bias=eps_sb)
    nc.vector.reciprocal(a_sb, var)
    nc.vector.tensor_tensor(c_sb, mus, a_sb, MUL)
    nc.vector.tensor_scalar_mul(c_sb, c_sb, -1.0)
    a_bf = small.tile([1, TOK], BF16, tag="lnbf", bufs=4, name=f"abf_{name}")
    c_bf = small.tile([1, TOK], BF16, tag="lnbf", bufs=4, name=f"cbf_{name}")
    nc.vector.tensor_copy(a_bf, a_sb)
    nc.vector.tensor_copy(c_bf, c_sb)
    aB = psum.tile([P, TOK], F32, tag="big", name=f"aB_{name}")
    cB = psum.tile([P, TOK], F32, tag="big", name=f"cB_{name}")
    nc.tensor.matmul(aB, lhsT=ones_row_bf, rhs=a_bf, start=True, stop=True)
    nc.tensor.matmul(cB, lhsT=ones_row_bf, rhs=c_bf, start=True, stop=True)
    aBs = small.tile([P, TOK], F32, tag="abs", name=f"aBs_{name}")
    cBs = small.tile([P, TOK], F32, tag="cbs", name=f"cBs_{name}")
    nc.vector.tensor_copy(aBs, aB)
    nc.vector.tensor_copy(cBs, cB)
    nc.vector.tensor_tensor(xs, xres,
                            aBs[:, None, :].to_broadcast((P, KS, TOK)), MUL)
    nc.vector.tensor_tensor(xs, xs,
                            cBs[:, None, :].to_broadcast((P, KS, TOK)),
                            mybir.AluOpType.add)
    nc.vector.tensor_copy(out_bf, xs)


# ===================== host side =====================

def _token_idx(parity):
    idx = []
    for m in range(S // BLK // 2):
        g = 2 * m + parity
        idx.extend(range(g * BLK, (g + 1) * BLK))
    return np.array(idx)


def _to_T_layout(xt):
    """[T, D] -> [P, D//P, T] (features on partitions)"""
    t = xt.shape[0]
    return np.ascontiguousarray(xt.T.reshape(KS, P, t).transpose(1, 0, 2))


def _make_masks(parity):
    cols = np.zeros((P, 4), np.float32)
    for r in range(2):
        for qh in range(2):
            for k in range(P):
                vis = (2 * (k // 64) + r) <= (2 * qh + parity)
                cols[k, r * 2 + qh] = 0.0 if vis else MASK_NEG
    return cols


def _prep_weights(in_proj_w, out_w, lin1_w, lin2_w):
    scale = np.float32(1.0 / np.sqrt(DH))
    wT = np.transpose(in_proj_w, (0, 2, 1)).astype(np.float32).copy()
    wT[:, :, :D] *= scale
    A = wT.reshape(L, KS, P, 3 * D).transpose(0, 2, 1, 3)          # [L,128,8,3072]
    wqkv = np.ascontiguousarray(
        A.reshape(L, P, KS, 6, 512).transpose(0, 3, 1, 2, 4)).astype(NPBF)
    Bm = np.transpose(out_w, (0, 2, 1)).reshape(L, KS, P, D).transpose(0, 2, 1, 3)
    wout = np.ascontiguousarray(
        Bm.reshape(L, P, KS, KS, P).transpose(0, 3, 1, 2, 4)).astype(NPBF)
    C = np.transpose(lin1_w, (0, 2, 1)).reshape(L, KS, P, DFF).transpose(0, 2, 1, 3)
    w1 = np.ascontiguousarray(
        C.reshape(L, P, KS, 8, 512).transpose(0, 3, 1, 2, 4)).astype(NPBF)
    E = np.transpose(lin2_w, (0, 2, 1)).reshape(L, FKS, P, D).transpose(0, 2, 1, 3)
    w2 = np.ascontiguousarray(
        E.reshape(L, P, FKS, KS, P).transpose(0, 3, 1, 2, 4)).astype(NPBF)
    return wqkv, wout, w1, w2


_PROGRAM = None


def _get_program():
    global _PROGRAM
    if _PROGRAM is None:
        _PROGRAM = build_program()
    return _PROGRAM


def make_in_maps(x, in_proj_w, in_proj_b, out_w, out_b, ln1_w, ln1_b,
                 lin1_w, lin1_b, lin2_w, lin2_b, ln2_w, ln2_b):
    x = np.asarray(x, np.float32)
    for bia in (in_proj_b, out_b, lin1_b, lin2_b, ln1_b, ln2_b):
        assert not np.asarray(bia).any(), "nonzero bias unsupported"
    assert (np.asarray(ln1_w) == 1).all() and (np.asarray(ln2_w) == 1).all()

    wqkv, wout, w1, w2 = _prep_weights(
        np.asarray(in_proj_w), np.asarray(out_w),
        np.asarray(lin1_w), np.asarray(lin2_w))

    idx = [_token_idx(0), _token_idx(1)]
    masks = [_make_masks(0), _make_masks(1)]
    in_maps = []
    for core in range(NCORES):
        b, par = core // 2, core % 2
        x_own = x[b][idx[par]]                       # [512, D]
        x_slot = np.concatenate([x[b][idx[0]], x[b][idx[1]]], 0)   # [1024, D]
        xs0 = _to_T_layout(x_own)
        in_maps.append({
            "xs0": xs0,
            "xbf0": xs0.astype(NPBF),
            "xfull0": _to_T_layout(x_slot).astype(NPBF),
            "maskc": masks[par],
            "wqkv": wqkv, "wout": wout, "w1": w1, "w2": w2,
        })
    return in_maps


def assemble_output(results):
    out = np.zeros((B, S, D), np.float32)
    idx = [_token_idx(0), _token_idx(1)]
    for core in range(NCORES):
        b, par = core // 2, core % 2
        y = results[core]["yT"]                      # [P, KS, TOK]
        xc = y.transpose(2, 1, 0).reshape(TOK, D)    # [tok, D]
        out[b][idx[par]] = xc
    return out


LAST_RESULTS = None


def kernel(**inputs):
    import os
    from concourse.bass_utils import run_bass_kernel_spmd
    global LAST_RESULTS
    nc = _get_program()
    in_maps = make_in_maps(**inputs)
    trace = bool(int(os.environ.get("KERNEL_TRACE", "0")))
    res = run_bass_kernel_spmd(nc, in_maps, core_ids=list(range(NCORES)),
                               trace=trace)
    LAST_RESULTS = res
    return assemble_output(res.results)
